# revision 1
# baseline (speedup 1.0000x reference)
"""Trainium2 Bass kernel for nn_Degrade: depthwise 13x13 blur + 4x downsample.

Reference computation (per sample, per channel):
  replicate-pad by 6, 13x13 cross-correlation with the per-sample kernel,
  stride-4 downsample: im [8,4,1024,1024] f32, kernel [8,1,13,13] f32
  -> out [8,4,256,256] f32.

Sharding: pure data parallel, one sample per NeuronCore (8 cores).

Per-core algorithm (single matmul pass, contraction over image rows):
  out[oy, ox] = sum_kx sum_y  Wb_kx[y, oy] * Impad[y, 4*ox + kx]
where Wb_kx[y, oy] = kernel[y - 4*oy, kx] is a banded matrix built on host.
Host prep (numpy):
  - replicate-pad image to [4, 1036, 1036]
  - polyphase-split x (x % 4) so every kx tap reads a CONTIGUOUS window of a
    phase plane (TensorE streams strided rhs at ~half rate, contiguous at
    1 col/cycle)
  - rows regrouped into 9 row-block tiles on two overlapping 128-row grids
    (oy tiles of {124,124,8}) so every matmul's rhs starts at partition 0
  - weights duplicated per row-block index j ([y, j, kx, 128]) so each j group
    is one contiguous DMA and every lhsT is a 128-col aligned block
  - everything cast to fp16 (PE streams fp16 at 1 col/cycle, halves DMA;
    PSUM accumulation is fp32)
Device: 12 PE warm-up matmuls bridge the DMA fill (HAM clock gate), then
208 matmuls [K=128, M=128, N=512] (PSUM rows 124-127 hold discarded partials)
+ 13 column-tiled matmul pairs for the last 8 output rows; N packs 2 channels
x 256 output columns (one PSUM bank). DMA issue alternates the two HWDGE
rings ordered by consumption deadline; the last block runs pair-outer so
PSUM drain/stores overlap the final matmuls.
"""
import numpy as np

import concourse.bacc as bacc
import concourse.mybir as mybir
import concourse.tile as tile
from concourse import bass_utils

KS = 13
PAD = 6
S = 4
B, C, H, W = 8, 4, 1024, 1024
OH = OW = 256
NPH = (W + 2 * PAD) // S  # 259
ROWL = C * S * NPH        # 4144
NROW = H + 2 * PAD        # 1036
MDT = mybir.dt.float16
NPDT = np.float16

# row-block grids: t0 tile rows, t1 tile rows (overlapping regrid), sliver rows
ROW_OFFS = [0, 128, 256, 384, 496, 624, 752, 880]
M_TILE = 124

_NC_CACHE = {}


def _host_pack_images(im: np.ndarray) -> np.ndarray:
    """im [8,4,1024,1024] f32 -> [8, 9, 128, ROWL] fp16 row-block tiles."""
    im_pad = np.pad(im, ((0, 0), (0, 0), (PAD, PAD), (PAD, PAD)), mode="edge")
    planes = im_pad.reshape(B, C, NROW, NPH, S).transpose(0, 1, 2, 4, 3)
    rows = (
        np.ascontiguousarray(planes.transpose(0, 2, 1, 3, 4))
        .reshape(B, NROW, ROWL)
        .astype(NPDT)
    )
    img = np.zeros((B, 9, 128, ROWL), NPDT)
    for g, y0 in enumerate(ROW_OFFS):
        img[:, g] = rows[:, y0 : y0 + 128]
    img[:, 8, :41] = rows[:, 992:1033]
    return img


def _host_pack_weights(kernel: np.ndarray) -> np.ndarray:
    """kernel [8,1,13,13] f32 -> [8, 128, 13*256] fp16 banded matrices.

    wfull[b, y, kx*256 + 128 + m] = kernel[b, 0, y - 4m, kx] (zero outside band).
    """
    ker = np.asarray(kernel, np.float32)[:, 0]  # [8,13,13]
    y = np.arange(128)[:, None]
    m = np.arange(256)[None, :] - 128
    ky = y - 4 * m
    valid = (ky >= 0) & (ky < KS)
    kyc = np.clip(ky, 0, KS - 1)
    wk = ker[:, kyc].transpose(0, 3, 1, 2)  # [8, 13(kx), 128(y), 256(m)]
    wfull = np.where(valid[None, None], wk, 0.0)  # [8, kx, y, 256]
    # per-j duplicated layout [8, y, j, kx, 128] so each j group is one
    # contiguous DMA and every lhsT is a 128-col aligned block
    wj = np.zeros((B, 128, 4, KS, 128), np.float32)
    for j in range(4):
        wj[:, :, j] = wfull.transpose(0, 2, 1, 3)[:, :, :, 128 - 32 * j : 256 - 32 * j]
    return np.ascontiguousarray(wj).reshape(B, 128, 4 * KS * 128).astype(NPDT)


def _build_nc():
    nc = bacc.Bacc("TRN2", target_bir_lowering=False, debug=False, num_devices=B)
    img_d = nc.dram_tensor("img", [9, 128, ROWL], MDT, kind="ExternalInput")
    w_d = nc.dram_tensor("wfull", [128, 4 * KS * 128], MDT, kind="ExternalInput")
    out_d = nc.dram_tensor("out", [OH, C * OW], mybir.dt.float32, kind="ExternalOutput")

    with tile.TileContext(nc) as tc:
        with (
            tc.tile_pool(name="wp", bufs=1) as wp,
            tc.tile_pool(name="ip", bufs=1) as ip,
            tc.tile_pool(name="op", bufs=4) as op,
            tc.tile_pool(name="ps", bufs=4, space="PSUM") as ps,
            tc.tile_pool(name="ps1", bufs=1, space="PSUM") as ps1,
        ):
            # weights: per-j slice groups, issued j0, j1 now; j2+j3 go out
            # after img1's second half (j=1's data deadline is tighter than
            # the j2/j3 weight deadlines)
            JG = KS * 128
            wall = wp.tile([128, 4 * KS * 128], MDT, tag="wall")
            nc.scalar.dma_start(wall[:, 0:JG], w_d.ap()[:, 0:JG])
            nc.scalar.dma_start(wall[:, JG : 2 * JG], w_d.ap()[:, JG : 2 * JG])

            # PE warm-up against the HAM clock gate while DMAs land
            warm = wp.tile([128, 512], MDT, tag="warm")
            nc.vector.memset(warm[:].bitcast(mybir.dt.uint16), 0)
            pwarm = ps1.tile([128, 512], mybir.dt.float32, tag="pwarm")
            for wi in range(12):
                nc.tensor.matmul(
                    pwarm[:], warm[:, 0:128], warm[:],
                    start=(wi == 0), stop=(wi == 11), skip_group_check=True,
                )

            imgs = {}
            half = ROWL // 2
            for g in range(9):
                tl = ip.tile([128, ROWL], MDT, tag=f"img{g}")
                eng = nc.sync if g % 2 == 0 else nc.scalar
                if g == 0:
                    # halves so the opening pair-outer MMs start sooner
                    eng.dma_start(tl[:, 0:half], img_d.ap()[g][:, 0:half])
                    eng.dma_start(tl[:, half:], img_d.ap()[g][:, half:])
                elif g == 1:
                    # split across BOTH rings: j=1's deadline is the tightest
                    nc.sync.dma_start(tl[:, 0:half], img_d.ap()[g][:, 0:half])
                    nc.scalar.dma_start(tl[:, half:], img_d.ap()[g][:, half:])
                    nc.scalar.dma_start(wall[:, 2 * JG :], w_d.ap()[:, 2 * JG :])
                elif g == 8:
                    # only 41 rows carry data; don't DMA the zero padding
                    eng.dma_start(tl[0:41, :], img_d.ap()[g][0:41, :])
                else:
                    eng.dma_start(tl[:], img_d.ap()[g])
                imgs[g] = tl

            M = M_TILE
            def do_tile(t):
                psums = []
                for pair in range(2):
                    acc = ps.tile([128, 512], mybir.dt.float32, tag="acc")
                    psums.append(acc)
                n_mm = 4 * KS
                pair_ct = [0, 0]
                for j in range(4):
                    g = 4 * t + j
                    rview = imgs[g][:].rearrange("p (c x) -> p c x", c=C)
                    # first block of the run: pair-outer so the opening 13 MMs
                    # need only wall[:, 0:256] + img0's first channel pair;
                    # last block: pair-outer so pair0's PSUM drains while
                    # pair1's final matmuls still stream
                    if (t == 0 and j == 0) or (t == 1 and j == 3):
                        order = [(kx, pair) for pair in range(2) for kx in range(KS)]
                    else:
                        order = [(kx, pair) for kx in range(KS) for pair in range(2)]
                    for kx, pair in order:
                        u, s = kx // S, kx % S
                        c0 = (j * KS + kx) * 128
                        off = s * NPH + u
                        rhs = rview[:, 2 * pair : 2 * pair + 2, off : off + 256]
                        # full M=128: psum rows M..127 accumulate partial
                        # (wrong) values for the next tile's first oy rows;
                        # they are never copied out. M=128 matmuls measure
                        # ~14 ns faster than M=124.
                        nc.tensor.matmul(
                            psums[pair][:, :], wall[:, c0 : c0 + 128], rhs,
                            start=(pair_ct[pair] == 0),
                            stop=(pair_ct[pair] == n_mm - 1),
                            skip_group_check=True,
                        )
                        pair_ct[pair] += 1
                for pair in range(2):
                    stage = op.tile([128, 512], mybir.dt.float32, tag="stage")
                    oeng = nc.sync if pair == 0 else nc.scalar
                    for h in range(2):
                        nc.vector.tensor_copy(
                            stage[0:M, 256 * h : 256 * h + 256],
                            psums[pair][0:M, 256 * h : 256 * h + 256],
                        )
                        oeng.dma_start(
                            out_d.ap()[
                                M * t : M * t + M,
                                512 * pair + 256 * h : 512 * pair + 256 * h + 256,
                            ],
                            stage[0:M, 256 * h : 256 * h + 256],
                        )

            do_tile(0)
            do_tile(1)
            # sliver: oy 248..255 (8 rows) from rows 992..1032; the two channel
            # pairs run CONCURRENTLY in different PE column groups
            acc2 = ps1.tile([64, 512], mybir.dt.float32, tag="acc2")
            rview = imgs[8][:].rearrange("p (c x) -> p c x", c=C)
            for kx in range(KS):
                u, s = kx // S, kx % S
                c0 = kx * 128
                off = s * NPH + u
                for pair in range(2):
                    rhs = rview[0:41, 2 * pair : 2 * pair + 2, off : off + 256]
                    nc.tensor.matmul(
                        acc2[32 * pair : 32 * pair + 8, :], wall[0:41, c0 : c0 + 8],
                        rhs,
                        start=(kx == 0), stop=(kx == KS - 1),
                        skip_group_check=True,
                        tile_position=(0, 32 * pair),
                    )
            stage2 = op.tile([8, 1024], mybir.dt.float32, tag="stage2")
            for pair in range(2):
                # pipeline: pair0's store drains while pair1's copy runs
                nc.vector.tensor_copy(
                    stage2[:, 512 * pair : 512 * pair + 512],
                    acc2[32 * pair : 32 * pair + 8, :],
                )
                oeng = nc.sync if pair == 0 else nc.scalar
                oeng.dma_start(
                    out_d.ap()[248:256, 512 * pair : 512 * pair + 512],
                    stage2[:, 512 * pair : 512 * pair + 512],
                )

    nc.compile()
    return nc


def get_nc():
    if "nc" not in _NC_CACHE:
        _NC_CACHE["nc"] = _build_nc()
    return _NC_CACHE["nc"]


def kernel(im, kernel, **run_kwargs):
    im = np.asarray(im, np.float32)
    kernel = np.asarray(kernel, np.float32)
    img = _host_pack_images(im)
    wfull = _host_pack_weights(kernel)
    nc = get_nc()
    in_maps = [{"img": img[b], "wfull": wfull[b]} for b in range(B)]
    res = bass_utils.run_bass_kernel_spmd(
        nc, in_maps, core_ids=list(range(B)), **run_kwargs
    )
    out = np.stack([r["out"] for r in res.results])  # [8, 256, 4*256]
    out = np.ascontiguousarray(out.reshape(B, OH, C, OW).transpose(0, 2, 1, 3))
    if run_kwargs:
        return out, res
    return out



# revision 6
# speedup vs baseline: 1.0018x; 1.0018x over previous
"""Trainium2 Bass kernel for nn_Degrade: depthwise 13x13 blur + 4x downsample.

Reference computation (per sample, per channel):
  replicate-pad by 6, 13x13 cross-correlation with the per-sample kernel,
  stride-4 downsample: im [8,4,1024,1024] f32, kernel [8,1,13,13] f32
  -> out [8,4,256,256] f32.

Sharding: pure data parallel, one sample per NeuronCore (8 cores).

Per-core algorithm (single matmul pass, contraction over image rows):
  out[oy, ox] = sum_kx sum_y  Wb_kx[y, oy] * Impad[y, 4*ox + kx]
where Wb_kx[y, oy] = kernel[y - 4*oy, kx] is a banded matrix built on host.

Structure (vs the M=124 + 8-row-sliver variant this replaces):
  - 2 output tiles of M=128 oy rows; 4 K=128 j-blocks each (rows 0..1024).
  - The 12 rows each tile misses (local rows 512..520, feeding local oy
    125..127) are handled by ONE kx-packed "edge" matmul per (tile, cg):
    partitions hold 9 rows x 13 kx pre-shifted windows, M=32 (cols 96..127
    of the psum, only the last 3 nonzero; tile_position 32-aligned).
    208 + 4 matmuls total instead of 208 + 26.
  - warm-up matmuls run at N=128 (not 512) to bridge the DMA fill cheaply.
  - host prep: polyphase-split x (x%4) so every kx tap reads a contiguous
    window; fp16 everywhere (PE streams fp16 at 1 col/cycle; PSUM fp32).
"""
import numpy as np

import concourse.bacc as bacc
import concourse.mybir as mybir
import concourse.tile as tile
from concourse import bass_utils

KS = 13
PAD = 6
S = 4
B, C, H, W = 8, 4, 1024, 1024
OH = OW = 256
NPH = (W + 2 * PAD) // S  # 259
ROWL = C * S * NPH        # 4144
NROW = H + 2 * PAD        # 1036
NE = 9 * KS               # 117 edge partitions
MDT = mybir.dt.float16
NPDT = np.float16

_NC_CACHE = {}


def _host_pack_images(im: np.ndarray):
    """im [8,4,1024,1024] f32 -> (img [8,8,128,ROWL], eimg [8,2,117,1024]) fp16.

    img: 8 j-blocks of 128 rows each (rows 0..1024), row layout [c, s, 259].
    eimg[b,t,q=kx*9+r9,:] = rows (512t+512+r9) window for kx, layout [c, 256].
    """
    im_pad = np.pad(im, ((0, 0), (0, 0), (PAD, PAD), (PAD, PAD)), mode="edge")
    planes = im_pad.reshape(B, C, NROW, NPH, S).transpose(0, 1, 2, 4, 3)
    rows = (
        np.ascontiguousarray(planes.transpose(0, 2, 1, 3, 4))
        .reshape(B, NROW, C, S, NPH)
        .astype(NPDT)
    )
    img = np.ascontiguousarray(rows[:, :1024].reshape(B, 8, 128, ROWL))
    eimg = np.zeros((B, 2, NE, C, OW), NPDT)
    for t in range(2):
        for kx in range(KS):
            u, s = kx // S, kx % S
            for r9 in range(9):
                eimg[:, t, kx * 9 + r9] = rows[:, 512 * t + 512 + r9, :, s, u : u + OW]
    return img, eimg.reshape(B, 2, NE, C * OW)


def _host_pack_weights(kernel: np.ndarray):
    """kernel [8,1,13,13] f32 -> (wall [8,128,4*13*128], wedge [8,117,32]) fp16.

    wall[b, p, (j,kx,m)] = ker[b, r-4m, kx] with r = 128j + p, 0 <= r-4m < 13.
    wedge[b, q=kx*9+r9, 61+i] = ker[b, r9+12-4i, kx] for out col 64+61+i
    (psum cols 64..124 zero, 125..127 = outputs).
    """
    ker = np.asarray(kernel, np.float32)[:, 0]  # [8,13,13]
    p = np.arange(128)[:, None]
    m = np.arange(128)[None, :]
    wall = np.zeros((B, 128, 4, KS, 128), np.float32)
    for j in range(4):
        ky = 128 * j + p - 4 * m
        valid = (ky >= 0) & (ky < KS)
        kyc = np.clip(ky, 0, KS - 1)
        # [8, 128p, 128m, 13kx] -> [8, p, kx, m]
        wj = np.where(valid[None, :, :, None], ker[:, kyc], 0.0)
        wall[:, :, j] = wj.transpose(0, 1, 3, 2)
    wedge = np.zeros((B, NE, 64), np.float32)
    for kx in range(KS):
        for r9 in range(9):
            for i in range(3):
                ky = r9 + 12 - 4 * i
                if 0 <= ky < KS:
                    wedge[:, kx * 9 + r9, 61 + i] = ker[:, ky, kx]
    return (
        np.ascontiguousarray(wall).reshape(B, 128, 4 * KS * 128).astype(NPDT),
        wedge.astype(NPDT),
    )


def _build_nc():
    nc = bacc.Bacc("TRN2", target_bir_lowering=False, debug=False, num_devices=B)
    img_d = nc.dram_tensor("img", [8, 128, ROWL], MDT, kind="ExternalInput")
    eimg_d = nc.dram_tensor("eimg", [2, NE, C * OW], MDT, kind="ExternalInput")
    w_d = nc.dram_tensor("wall", [128, 4 * KS * 128], MDT, kind="ExternalInput")
    we_d = nc.dram_tensor("wedge", [NE, 64], MDT, kind="ExternalInput")
    out_d = nc.dram_tensor("out", [OH, C * OW], mybir.dt.float32, kind="ExternalOutput")

    with tile.TileContext(nc) as tc:
        with (
            tc.tile_pool(name="wp", bufs=1) as wp,
            tc.tile_pool(name="ip", bufs=1) as ip,
            tc.tile_pool(name="op", bufs=4) as op,
            tc.tile_pool(name="ps", bufs=4, space="PSUM") as ps,
            tc.tile_pool(name="ps1", bufs=1, space="PSUM") as ps1,
        ):
            JG = KS * 128
            half = ROWL // 2  # 2072 = [c0,c1] channels of a row
            wall = wp.tile([128, 4 * KS * 128], MDT, tag="wall")
            wedge = wp.tile([NE, 64], MDT, tag="wedge")
            eimgs = wp.tile([NE, 2 * C * OW], MDT, tag="eimgs")

            # --- DMA issue, ordered by consumption deadline -------------
            # ring A (sync): image c01 halves of g0..g3 first (cg0 of t0),
            # ring B (scalar): weights j0 first, then c01 of g4..g7.
            imgs = {}
            for g in range(8):
                tl = ip.tile([128, ROWL], MDT, tag=f"img{g}")
                imgs[g] = tl
            # first matmul needs wall[j0,kx0] + g0's [c01, s0] plane; kx
            # iterates s-major so s-planes of g0 gate successive MM quads
            nc.scalar.dma_start(wall[:, 0 : 4 * 128], w_d.ap()[:, 0 : 4 * 128])
            g0v = imgs[0][:].rearrange("p (c x) -> p c x", c=C)
            g0d = img_d.ap()[0].rearrange("p (c x) -> p c x", c=C)
            for s in range(S):
                nc.sync.dma_start(
                    g0v[:, 0:2, s * NPH : (s + 1) * NPH],
                    g0d[:, 0:2, s * NPH : (s + 1) * NPH],
                )
            nc.scalar.dma_start(wall[:, 4 * 128 : JG], w_d.ap()[:, 4 * 128 : JG])
            nc.sync.dma_start(imgs[1][:, 0:half], img_d.ap()[1][:, 0:half])
            nc.scalar.dma_start(wall[:, JG : 2 * JG], w_d.ap()[:, JG : 2 * JG])
            nc.sync.dma_start(imgs[2][:, 0:half], img_d.ap()[2][:, 0:half])
            nc.scalar.dma_start(wall[:, 2 * JG :], w_d.ap()[:, 2 * JG :])
            nc.scalar.dma_start(wedge[:], we_d.ap())
            nc.sync.dma_start(imgs[3][:, 0:half], img_d.ap()[3][:, 0:half])
            nc.scalar.dma_start(eimgs[:, 0 : C * OW], eimg_d.ap()[0])
            # cg1 of t0 + cg0 of t1 interleaved across rings
            nc.sync.dma_start(imgs[0][:, half:], img_d.ap()[0][:, half:])
            nc.scalar.dma_start(imgs[1][:, half:], img_d.ap()[1][:, half:])
            nc.sync.dma_start(imgs[2][:, half:], img_d.ap()[2][:, half:])
            nc.scalar.dma_start(imgs[3][:, half:], img_d.ap()[3][:, half:])
            nc.sync.dma_start(imgs[4][:, 0:half], img_d.ap()[4][:, 0:half])
            nc.scalar.dma_start(imgs[5][:, 0:half], img_d.ap()[5][:, 0:half])
            nc.sync.dma_start(imgs[6][:, 0:half], img_d.ap()[6][:, 0:half])
            nc.scalar.dma_start(imgs[7][:, 0:half], img_d.ap()[7][:, 0:half])
            nc.sync.dma_start(eimgs[:, C * OW :], eimg_d.ap()[1])
            nc.scalar.dma_start(imgs[4][:, half:], img_d.ap()[4][:, half:])
            nc.sync.dma_start(imgs[5][:, half:], img_d.ap()[5][:, half:])
            nc.scalar.dma_start(imgs[6][:, half:], img_d.ap()[6][:, half:])
            nc.sync.dma_start(imgs[7][:, half:], img_d.ap()[7][:, half:])

            # --- PE warm-up (N=128) against the HAM clock gate ----------
            warm = wp.tile([128, 128], MDT, tag="warm")
            nc.vector.memset(warm[:].bitcast(mybir.dt.uint16), 0)
            pwarm = ps1.tile([128, 128], mybir.dt.float32, tag="pwarm")
            for wi in range(10):
                nc.tensor.matmul(
                    pwarm[:], warm[:], warm[:],
                    start=(wi == 0), stop=(wi == 9), skip_group_check=True,
                )

            # --- main loop: 4 groups of (52 banded + 1 edge) matmuls ----
            KX_ORDER = [0, 4, 8, 12, 1, 5, 9, 2, 6, 10, 3, 7, 11]  # s-major

            def do_group(t, cg, last):
                acc = ps.tile([128, 512], mybir.dt.float32, tag="acc")
                for j in range(4):
                    rview = imgs[4 * t + j][:].rearrange("p (c x) -> p c x", c=C)
                    for i, kx in enumerate(KX_ORDER):
                        u, s = kx // S, kx % S
                        c0 = (j * KS + kx) * 128
                        off = s * NPH + u
                        rhs = rview[:, 2 * cg : 2 * cg + 2, off : off + 256]
                        nc.tensor.matmul(
                            acc[:, :], wall[:, c0 : c0 + 128], rhs,
                            start=(j == 0 and i == 0), stop=False,
                            skip_group_check=True,
                        )
                # edge: rows 512t+512..+520, all 13 kx packed in K; writes
                # psum cols 64..127 (only 125..127 nonzero), 64-aligned.
                erhs = eimgs[:, t * C * OW + 512 * cg : t * C * OW + 512 * cg + 512]
                nc.tensor.matmul(
                    acc[64:128, :], wedge[:, :], erhs,
                    start=False, stop=True, skip_group_check=True,
                    tile_position=(0, 64),
                )
                # drain: 2 x 256-col chunks (4 x 128 for the last group)
                stage = op.tile([128, 512], mybir.dt.float32, tag="stage")
                nchunk = 4 if last else 2
                w_ = 512 // nchunk
                for h in range(nchunk):
                    nc.vector.tensor_copy(
                        stage[:, w_ * h : w_ * h + w_],
                        acc[:, w_ * h : w_ * h + w_],
                    )
                    oeng = nc.sync if h % 2 == 0 else nc.scalar
                    oeng.dma_start(
                        out_d.ap()[
                            128 * t : 128 * t + 128,
                            512 * cg + w_ * h : 512 * cg + w_ * h + w_,
                        ],
                        stage[:, w_ * h : w_ * h + w_],
                    )

            do_group(0, 0, False)
            do_group(0, 1, False)
            do_group(1, 0, False)
            do_group(1, 1, True)

    nc.compile()
    return nc


def get_nc():
    if "nc" not in _NC_CACHE:
        _NC_CACHE["nc"] = _build_nc()
    return _NC_CACHE["nc"]


def kernel(im, kernel, **run_kwargs):
    im = np.asarray(im, np.float32)
    kernel = np.asarray(kernel, np.float32)
    img, eimg = _host_pack_images(im)
    wall, wedge = _host_pack_weights(kernel)
    nc = get_nc()
    in_maps = [
        {"img": img[b], "eimg": eimg[b], "wall": wall[b], "wedge": wedge[b]}
        for b in range(B)
    ]
    res = bass_utils.run_bass_kernel_spmd(
        nc, in_maps, core_ids=list(range(B)), **run_kwargs
    )
    out = np.stack([r["out"] for r in res.results])  # [8, 256, 4*256]
    out = np.ascontiguousarray(out.reshape(B, OH, C, OW).transpose(0, 2, 1, 3))
    if run_kwargs:
        return out, res
    return out


# revision 8
# speedup vs baseline: 1.0199x; 1.0181x over previous
"""Trainium2 Bass kernel for nn_Degrade: depthwise 13x13 blur + 4x downsample.

Reference computation (per sample, per channel):
  replicate-pad by 6, 13x13 cross-correlation with the per-sample kernel,
  stride-4 downsample: im [8,4,1024,1024] f32, kernel [8,1,13,13] f32
  -> out [8,4,256,256] f32.

Sharding: pure data parallel, one sample per NeuronCore (8 cores).

Per-core algorithm (single matmul pass, contraction over image rows):
  out[oy, ox] = sum_kx sum_y  Wb_kx[y, oy] * Impad[y, 4*ox + kx]
where Wb_kx[y, oy] = kernel[y - 4*oy, kx] is a banded matrix built on host.

Structure (vs the M=124 + 8-row-sliver variant this replaces):
  - 2 output tiles of M=128 oy rows; 4 K=128 j-blocks each (rows 0..1024).
  - The 12 rows each tile misses (local rows 512..520, feeding local oy
    125..127) are handled by ONE kx-packed "edge" matmul per (tile, cg):
    partitions hold 9 rows x 13 kx pre-shifted windows, M=32 (cols 96..127
    of the psum, only the last 3 nonzero; tile_position 32-aligned).
    208 + 4 matmuls total instead of 208 + 26.
  - warm-up matmuls run at N=128 (not 512) to bridge the DMA fill cheaply.
  - host prep: polyphase-split x (x%4) so every kx tap reads a contiguous
    window; fp16 everywhere (PE streams fp16 at 1 col/cycle; PSUM fp32).
"""
import numpy as np

import concourse.bacc as bacc
import concourse.mybir as mybir
import concourse.tile as tile
from concourse import bass_utils

KS = 13
PAD = 6
S = 4
B, C, H, W = 8, 4, 1024, 1024
OH = OW = 256
NPH = (W + 2 * PAD) // S  # 259
ROWL = C * S * NPH        # 4144
NROW = H + 2 * PAD        # 1036
NE = 9 * KS               # 117 edge partitions
MDT = mybir.dt.float16
NPDT = np.float16

_NC_CACHE = {}


def _host_pack_images(im: np.ndarray):
    """im [8,4,1024,1024] f32 -> (img [8,8,128,ROWL], eimg [8,2,117,1024]) fp16.

    img: 8 j-blocks of 128 rows each (rows 0..1024), row layout [c, s, 259].
    eimg[b,t,q=kx*9+r9,:] = rows (512t+512+r9) window for kx, layout [c, 256].
    """
    im_pad = np.pad(im, ((0, 0), (0, 0), (PAD, PAD), (PAD, PAD)), mode="edge")
    planes = im_pad.reshape(B, C, NROW, NPH, S).transpose(0, 1, 2, 4, 3)
    rows = (
        np.ascontiguousarray(planes.transpose(0, 2, 1, 3, 4))
        .reshape(B, NROW, C, S, NPH)
        .astype(NPDT)
    )
    img = np.ascontiguousarray(rows[:, :1024].reshape(B, 8, 128, ROWL))
    eimg = np.zeros((B, 2, NE, C, OW), NPDT)
    for t in range(2):
        for kx in range(KS):
            u, s = kx // S, kx % S
            for r9 in range(9):
                eimg[:, t, kx * 9 + r9] = rows[:, 512 * t + 512 + r9, :, s, u : u + OW]
    return img, eimg.reshape(B, 2, NE, C * OW)


def _host_pack_weights(kernel: np.ndarray):
    """kernel [8,1,13,13] f32 -> (wall [8,128,4*13*128], wedge [8,117,32]) fp16.

    wall[b, p, (j,kx,m)] = ker[b, r-4m, kx] with r = 128j + p, 0 <= r-4m < 13.
    wedge[b, q=kx*9+r9, 61+i] = ker[b, r9+12-4i, kx] for out col 64+61+i
    (psum cols 64..124 zero, 125..127 = outputs).
    """
    ker = np.asarray(kernel, np.float32)[:, 0]  # [8,13,13]
    p = np.arange(128)[:, None]
    m = np.arange(128)[None, :]
    wall = np.zeros((B, 128, 4, KS, 128), np.float32)
    for j in range(4):
        ky = 128 * j + p - 4 * m
        valid = (ky >= 0) & (ky < KS)
        kyc = np.clip(ky, 0, KS - 1)
        # [8, 128p, 128m, 13kx] -> [8, p, kx, m]
        wj = np.where(valid[None, :, :, None], ker[:, kyc], 0.0)
        wall[:, :, j] = wj.transpose(0, 1, 3, 2)
    wedge = np.zeros((B, NE, 64), np.float32)
    for kx in range(KS):
        for r9 in range(9):
            for i in range(3):
                ky = r9 + 12 - 4 * i
                if 0 <= ky < KS:
                    wedge[:, kx * 9 + r9, 61 + i] = ker[:, ky, kx]
    return (
        np.ascontiguousarray(wall).reshape(B, 128, 4 * KS * 128).astype(NPDT),
        wedge.astype(NPDT),
    )


def _build_nc():
    nc = bacc.Bacc("TRN2", target_bir_lowering=False, debug=False, num_devices=B)
    img_d = nc.dram_tensor("img", [8, 128, ROWL], MDT, kind="ExternalInput")
    eimg_d = nc.dram_tensor("eimg", [2, NE, C * OW], MDT, kind="ExternalInput")
    w_d = nc.dram_tensor("wall", [128, 4 * KS * 128], MDT, kind="ExternalInput")
    we_d = nc.dram_tensor("wedge", [NE, 64], MDT, kind="ExternalInput")
    out_d = nc.dram_tensor("out", [OH, C * OW], mybir.dt.float32, kind="ExternalOutput")

    with tile.TileContext(nc) as tc:
        with (
            tc.tile_pool(name="wp", bufs=1) as wp,
            tc.tile_pool(name="ip", bufs=1) as ip,
            tc.tile_pool(name="op", bufs=4) as op,
            tc.tile_pool(name="ps", bufs=4, space="PSUM") as ps,
            tc.tile_pool(name="ps1", bufs=1, space="PSUM") as ps1,
        ):
            JG = KS * 128
            half = ROWL // 2  # 2072 = [c0,c1] channels of a row
            wall = wp.tile([128, 4 * KS * 128], MDT, tag="wall")
            wedge = wp.tile([NE, 64], MDT, tag="wedge")
            eimgs = wp.tile([NE, 2 * C * OW], MDT, tag="eimgs")

            # --- DMA issue, ordered by consumption deadline -------------
            # ring A (sync): image c01 halves of g0..g3 first (cg0 of t0),
            # ring B (scalar): weights j0 first, then c01 of g4..g7.
            imgs = {}
            for g in range(8):
                tl = ip.tile([128, ROWL], MDT, tag=f"img{g}")
                imgs[g] = tl
            # first matmul needs wall[j0] + g0's c01 half; contiguous halves
            # keep the DMA rings at full rate (strided pieces measured slow)
            nc.scalar.dma_start(wall[:, 0:JG], w_d.ap()[:, 0:JG])
            nc.sync.dma_start(imgs[0][:, 0:half], img_d.ap()[0][:, 0:half])
            nc.sync.dma_start(imgs[1][:, 0:half], img_d.ap()[1][:, 0:half])
            nc.scalar.dma_start(wall[:, JG : 2 * JG], w_d.ap()[:, JG : 2 * JG])
            nc.sync.dma_start(imgs[2][:, 0:half], img_d.ap()[2][:, 0:half])
            nc.scalar.dma_start(wall[:, 2 * JG :], w_d.ap()[:, 2 * JG :])
            nc.scalar.dma_start(wedge[:], we_d.ap())
            nc.sync.dma_start(imgs[3][:, 0:half], img_d.ap()[3][:, 0:half])
            nc.scalar.dma_start(eimgs[:, 0 : C * OW], eimg_d.ap()[0])
            # cg1 of t0 + cg0 of t1 interleaved across rings
            nc.sync.dma_start(imgs[0][:, half:], img_d.ap()[0][:, half:])
            nc.scalar.dma_start(imgs[1][:, half:], img_d.ap()[1][:, half:])
            nc.sync.dma_start(imgs[2][:, half:], img_d.ap()[2][:, half:])
            nc.scalar.dma_start(imgs[3][:, half:], img_d.ap()[3][:, half:])
            nc.sync.dma_start(imgs[4][:, 0:half], img_d.ap()[4][:, 0:half])
            nc.scalar.dma_start(imgs[5][:, 0:half], img_d.ap()[5][:, 0:half])
            nc.sync.dma_start(imgs[6][:, 0:half], img_d.ap()[6][:, 0:half])
            nc.scalar.dma_start(imgs[7][:, 0:half], img_d.ap()[7][:, 0:half])
            nc.sync.dma_start(eimgs[:, C * OW :], eimg_d.ap()[1])
            nc.scalar.dma_start(imgs[4][:, half:], img_d.ap()[4][:, half:])
            nc.sync.dma_start(imgs[5][:, half:], img_d.ap()[5][:, half:])
            nc.scalar.dma_start(imgs[6][:, half:], img_d.ap()[6][:, half:])
            nc.sync.dma_start(imgs[7][:, half:], img_d.ap()[7][:, half:])

            # --- PE warm-up against the HAM clock gate; sized to end when
            # the DMA fill can sustain the real matmul stream (~2.8us) ----
            warm = wp.tile([128, 512], MDT, tag="warm")
            nc.vector.memset(warm[:].bitcast(mybir.dt.uint16), 0)
            pwarm = ps1.tile([128, 512], mybir.dt.float32, tag="pwarm")
            for wi in range(12):
                nc.tensor.matmul(
                    pwarm[:], warm[:, 0:128], warm[:],
                    start=(wi == 0), stop=(wi == 11), skip_group_check=True,
                )

            # --- main loop: 4 groups of (52 banded + 1 edge) matmuls ----
            KX_ORDER = [0, 4, 8, 12, 1, 5, 9, 2, 6, 10, 3, 7, 11]  # s-major

            def do_group(t, cg, last):
                acc = ps.tile([128, 512], mybir.dt.float32, tag="acc")
                for j in range(4):
                    rview = imgs[4 * t + j][:].rearrange("p (c x) -> p c x", c=C)
                    for i, kx in enumerate(KX_ORDER):
                        u, s = kx // S, kx % S
                        c0 = (j * KS + kx) * 128
                        off = s * NPH + u
                        rhs = rview[:, 2 * cg : 2 * cg + 2, off : off + 256]
                        nc.tensor.matmul(
                            acc[:, :], wall[:, c0 : c0 + 128], rhs,
                            start=(j == 0 and i == 0), stop=False,
                            skip_group_check=True,
                        )
                # edge: rows 512t+512..+520, all 13 kx packed in K; writes
                # psum cols 64..127 (only 125..127 nonzero), 64-aligned.
                erhs = eimgs[:, t * C * OW + 512 * cg : t * C * OW + 512 * cg + 512]
                nc.tensor.matmul(
                    acc[64:128, :], wedge[:, :], erhs,
                    start=False, stop=True, skip_group_check=True,
                    tile_position=(0, 64),
                )
                # drain: 2 x 256-col chunks (4 x 128 for the last group)
                stage = op.tile([128, 512], mybir.dt.float32, tag="stage")
                nchunk = 4 if last else 2
                w_ = 512 // nchunk
                for h in range(nchunk):
                    nc.vector.tensor_copy(
                        stage[:, w_ * h : w_ * h + w_],
                        acc[:, w_ * h : w_ * h + w_],
                    )
                    oeng = nc.sync if h % 2 == 0 else nc.scalar
                    oeng.dma_start(
                        out_d.ap()[
                            128 * t : 128 * t + 128,
                            512 * cg + w_ * h : 512 * cg + w_ * h + w_,
                        ],
                        stage[:, w_ * h : w_ * h + w_],
                    )

            do_group(0, 0, False)
            do_group(0, 1, False)
            do_group(1, 0, False)
            do_group(1, 1, True)

    nc.compile()
    return nc


def get_nc():
    if "nc" not in _NC_CACHE:
        _NC_CACHE["nc"] = _build_nc()
    return _NC_CACHE["nc"]


def kernel(im, kernel, **run_kwargs):
    im = np.asarray(im, np.float32)
    kernel = np.asarray(kernel, np.float32)
    img, eimg = _host_pack_images(im)
    wall, wedge = _host_pack_weights(kernel)
    nc = get_nc()
    in_maps = [
        {"img": img[b], "eimg": eimg[b], "wall": wall[b], "wedge": wedge[b]}
        for b in range(B)
    ]
    res = bass_utils.run_bass_kernel_spmd(
        nc, in_maps, core_ids=list(range(B)), **run_kwargs
    )
    out = np.stack([r["out"] for r in res.results])  # [8, 256, 4*256]
    out = np.ascontiguousarray(out.reshape(B, OH, C, OW).transpose(0, 2, 1, 3))
    if run_kwargs:
        return out, res
    return out
